# revision 21
# baseline (speedup 1.0000x reference)
"""Cross-attention Trainium2 kernel (8 NeuronCores, batch x head-group sharded).

Problem (hardcoded): B=4, T=2048, M=4096, D=512, H=8, Dh=64, fp32 I/O.
Core c = (batch b = c//2, head-group g = c%2): heads [4g, 4g+4), full T.
Wq/Wk/Wv column-sharded, Wo row-sharded; the two partial O-projections of a
batch are summed at unshard time (host), per the row-parallel decomposition.

v6 pipeline (per core):
  A. x/mem shipped as f16 from host; DMA-transpose (X-bar) straight into
     persistent xT/memT SBUF tiles -- no PE transposes, no converts, no
     PSUM bounce for the input side.
  B. K^T / V / Q^T projections off memT/xT. Wk is pre-scaled by log2(e)*8
     host-side so PSUM scores are directly in e4m3-bits scale. V is stored
     as fp8 e4m3 (VX8) with a fused ones-column for the softmax denominator.
  C. Attention per (t-chunk, head-pair), software-pipelined. exp produces
     A' = exp(s-2) in fp8 e4m3 (softmax scale-invariant):
       ScalarE groups: native Exp(psc*scale - 2) -> e4m3 convert
       DVE groups: one-pass Schraudolph: i8(psc + B8) max 0 == e4m3 bits
     AV matmuls run fully in fp8 (4x faster LDWEIGHTS via FWL).
  D. O-projection folded into the attention loop (tci-outer order) so PE/
     DVE/Act tail work pipelines under later t-chunks' attention.
"""

import numpy as np

B, T, M, D = 4, 2048, 4096, 512
H, DH = 8, 64
HL = 4                # local heads per core
N_CORES = 8
ND = D // 128         # 4 d_in tiles
NDO = HL * DH // 128  # 2 d_out tiles
NM = M // 128         # 32 m-tiles
NTT = T // 128        # 16 t-tiles
TCH = 256             # attention t-chunk
NTC = T // TCH        # 8 t-chunks
MGROUPS = [2] * 16    # m-tiles per PSUM score group (sum=32)
DVE_GROUPS = {1, 3, 5, 7, 9, 11, 13}  # exp groups offloaded to DVE

# Wk is scaled by log2(e) host-side: psc = 11.5416*s where s = q.k/8.
# ScalarE groups emit A' = exp(s - CA) as e4m3 (safe: scores span +-7.7 sigma,
# e4m3 inf would need s > CA+5.48). DVE groups emit A'' = exp(s - CD) as
# f16 Schraudolph bits (f16 covers +-10 sigma). The scale mismatch
# e^(CA-CD) is cancelled by scaling the DVE-group V tiles (and their
# denominator ones-column) by e^(CD-CA).
KSCALE = float(np.log2(np.e))            # psc = q.k * log2e = 11.5416*s
ASCALE = float(1.0 / (8.0 * np.log2(np.e)))  # Exp arg scale for ScalarE path
CA, CD = 3.5, 0.0
SCH_A16 = 128.0                          # f16 bits = psc*128 + B16
SCH_B16 = float(1024.0 * (15.0 - CD * np.log2(np.e)) - 45.0)
VSCALE_D = float(np.exp(CD - CA))        # V rescale for DVE-group m-tiles

_CACHE = {}


def _build(reps=1):
    import concourse.bacc as bacc
    import concourse.mybir as mybir
    import concourse.tile as tile
    from contextlib import ExitStack

    f32 = mybir.dt.float32
    f16 = mybir.dt.float16
    f8 = mybir.dt.float8e4
    i16 = mybir.dt.int16
    AF = mybir.ActivationFunctionType
    ALU = mybir.AluOpType

    nc = bacc.Bacc("TRN2", target_bir_lowering=False, debug=False,
                   num_devices=N_CORES)

    x_d = nc.dram_tensor("x16T", [D, T], f16, kind="ExternalInput").ap()
    mem_d = nc.dram_tensor("mem16T", [D, M], f16, kind="ExternalInput").ap()
    wq_d = nc.dram_tensor("Wq", [D, HL * DH], f16, kind="ExternalInput").ap()
    wk_d = nc.dram_tensor("Wk", [D, HL * DH], f16, kind="ExternalInput").ap()
    wv_d = nc.dram_tensor("Wv", [D, HL * DH], f16, kind="ExternalInput").ap()
    wo_d = nc.dram_tensor("Wo", [HL * DH, D], f16, kind="ExternalInput").ap()
    bqT_d = nc.dram_tensor("bqT", [128, NDO], f32, kind="ExternalInput").ap()
    identg_d = nc.dram_tensor("identg", [128, 128], f16, kind="ExternalInput").ap()
    out_d = nc.dram_tensor("out", [T, D], f32, kind="ExternalOutput").ap()

    with tile.TileContext(nc) as tc, ExitStack() as top:
        const = top.enter_context(tc.tile_pool(name="const", bufs=1))
        persist = top.enter_context(tc.tile_pool(name="persist", bufs=1))

        identg = const.tile([128, 128], f16, tag="identg")
        nc.sync.dma_start(identg[:], identg_d[:])
        bqT = const.tile([128, NDO], f32, tag="bqT")
        biasca = const.tile([128, 1], f32, tag="biasca")
        nc.vector.memset(biasca[:], -CA)
        w_sb = {nm: [const.tile([128, HL * DH], f16, tag=f"W{nm}{di}", name=f"W{nm}{di}")
                     for di in range(ND)] for nm in ("q", "k", "v")}
        wo_sb = [const.tile([128, D], f16, tag=f"Wo{di}", name=f"Wo{di}")
                 for di in range(NDO)]
        w_dram = {"q": wq_d, "k": wk_d, "v": wv_d}

        def load_w(nm):
            for di in range(ND):
                nc.sync.dma_start(w_sb[nm][di][:], w_dram[nm][di * 128:(di + 1) * 128, :])

        # persistent activations
        memT = [persist.tile([128, M], f16, tag=f"memT{di}", name=f"memT{di}")
                for di in range(ND)]
        xT = [persist.tile([128, T], f16, tag=f"xT{di}", name=f"xT{di}")
              for di in range(ND)]
        KT = [persist.tile([128, M], f16, tag=f"KT{do}", name=f"KT{do}") for do in range(NDO)]
        QT = [persist.tile([128, T], f16, tag=f"QT{do}", name=f"QT{do}") for do in range(NDO)]
        VX = [persist.tile([128, HL * 65],
                           f16 if (mt // 2) in DVE_GROUPS else f8,
                           tag=f"VX{mt}", name=f"VX{mt}") for mt in range(NM)]
        ZSB = [persist.tile([128, HL * DH], f16, tag=f"Z{tt}", name=f"Z{tt}") for tt in range(NTT)]

        for _rep in range(reps):
            # ---- Phase A/B: chunked DMA of host-pre-transposed f16 inputs ----
            # 512-col chunks so the first K-projection starts after ~4 small
            # DMAs instead of the whole 6MB.
            if _rep == 0:
                load_w("k")
                load_w("v")
                load_w("q")
                nc.sync.dma_start(bqT[:], bqT_d[:])
                for di in range(NDO):
                    nc.sync.dma_start(wo_sb[di][:], wo_d[di * 128:(di + 1) * 128, :])
            for mc in range(M // 512):
                for di in range(ND):
                    nc.sync.dma_start(
                        memT[di][:, mc * 512:(mc + 1) * 512],
                        mem_d[di * 128:(di + 1) * 128, mc * 512:(mc + 1) * 512])
            for tcx in range(T // 512):
                for di in range(ND):
                    nc.sync.dma_start(
                        xT[di][:, tcx * 512:(tcx + 1) * 512],
                        x_d[di * 128:(di + 1) * 128, tcx * 512:(tcx + 1) * 512])

            with (
                tc.tile_pool(name="pk", bufs=2, space="PSUM") as pk_pool,
                tc.tile_pool(name="pv", bufs=2, space="PSUM") as pv_pool,
                tc.tile_pool(name="pq", bufs=2, space="PSUM") as pq_pool,
            ):
                for mc in range(M // 512):  # 8 chunks of 512 mem rows
                    # K^T chunk [dout-128, 512 m-cols]; bk dropped (softmax shift)
                    for do in range(NDO):
                        pk = pk_pool.tile([128, 512], f32, tag="pk")
                        for di in range(ND):
                            nc.tensor.matmul(
                                pk[:], w_sb["k"][di][:, do * 128:(do + 1) * 128],
                                memT[di][:, mc * 512:(mc + 1) * 512],
                                start=(di == 0), stop=(di == ND - 1))
                        if do == 0:
                            nc.scalar.copy(KT[do][:, mc * 512:(mc + 1) * 512], pk[:])
                        else:
                            nc.vector.tensor_copy(KT[do][:, mc * 512:(mc + 1) * 512], pk[:])
                    # V chunk: per m-tile [m-128, HL*64] -> VX fp8 with ones cols
                    for j in range(4):
                        mt = mc * 4 + j
                        pv = pv_pool.tile([128, HL * DH], f32, tag="pv")
                        for di in range(ND):
                            nc.tensor.matmul(pv[:], memT[di][:, mt * 128:(mt + 1) * 128],
                                             w_sb["v"][di][:],
                                             start=(di == 0), stop=(di == ND - 1))
                        vx3 = VX[mt][:].rearrange("p (h c) -> p h c", h=HL)
                        if (mt // 2) in DVE_GROUPS:
                            nc.scalar.mul(vx3[:, :, 0:64],
                                          pv[:].rearrange("p (h c) -> p h c", h=HL),
                                          VSCALE_D)
                            nc.vector.memset(vx3[:, :, 64:65], VSCALE_D)
                        else:
                            nc.scalar.copy(vx3[:, :, 0:64],
                                           pv[:].rearrange("p (h c) -> p h c", h=HL))
                            nc.vector.memset(vx3[:, :, 64:65], 1.0)
                for tcx in range(T // 512):  # 4 chunks -> Q^T (+bq on DVE copy)
                    for do in range(NDO):
                        pq = pq_pool.tile([128, 512], f32, tag="pq")
                        for di in range(ND):
                            nc.tensor.matmul(
                                pq[:], w_sb["q"][di][:, do * 128:(do + 1) * 128],
                                xT[di][:, tcx * 512:(tcx + 1) * 512],
                                start=(di == 0), stop=(di == ND - 1))
                        nc.vector.tensor_scalar_add(
                            QT[do][:, tcx * 512:(tcx + 1) * 512], pq[:],
                            bqT[:, do:do + 1])

            # ---- Phase C: attention, software-pipelined ----
            with (
                tc.tile_pool(name="psc", bufs=3, space="PSUM") as psc_pool,
                tc.tile_pool(name="pz", bufs=2, space="PSUM") as pz_pool,
                tc.tile_pool(name="esb8", bufs=20) as e8_pool,
                tc.tile_pool(name="esb16", bufs=16) as e16_pool,
                tc.tile_pool(name="rcp", bufs=2) as rcp_pool,
            ):
                HB = 2 * TCH  # per-head column block inside a score group

                def emit_scores(hp, tci):
                    egroups = []
                    mt0 = 0
                    for gi, msz in enumerate(MGROUPS):
                        psc = psc_pool.tile([128, 2 * 512], f32, tag="psc")
                        for j in range(msz):
                            mt = mt0 + j
                            for hl in range(2):
                                nc.tensor.matmul(
                                    psc[:, hl * HB + j * TCH: hl * HB + (j + 1) * TCH],
                                    KT[hp][hl * 64:(hl + 1) * 64,
                                           mt * 128:(mt + 1) * 128],
                                    QT[hp][hl * 64:(hl + 1) * 64,
                                           tci * TCH:(tci + 1) * TCH],
                                    start=True, stop=True,
                                    tile_position=(hl * 64, 0))
                        if gi in DVE_GROUPS:
                            # f16-bits Schraudolph: i16(psc*128 + B16)
                            esb = e16_pool.tile([128, 2 * 512], f16, tag="esb16")
                            nc.vector.tensor_scalar(
                                esb[:].bitcast(i16), psc[:],
                                SCH_A16, SCH_B16, op0=ALU.mult, op1=ALU.add)
                        else:
                            esb = e8_pool.tile([128, 2 * 512], f8, tag="esb8")
                            nc.scalar.activation(esb[:], psc[:], AF.Exp,
                                                 scale=ASCALE, bias=biasca[:])
                        egroups.append((mt0, msz, esb))
                        mt0 += msz
                    return egroups

                def emit_av_norm(hp, tci, egroups):
                    pz = pz_pool.tile([128, 4 * 65], f32, tag="pz")
                    for ts in range(2):
                        for hl in range(2):
                            for g0, gsz, esb in egroups:
                                for j in range(gsz):
                                    mt = g0 + j
                                    nc.tensor.matmul(
                                        pz[:, (ts * 2 + hl) * 65:(ts * 2 + hl) * 65 + 65],
                                        esb[:, hl * HB + j * TCH + ts * 128:
                                            hl * HB + j * TCH + (ts + 1) * 128],
                                        VX[mt][:].rearrange("p (h c) -> p h c", h=HL)[:, 2 * hp + hl, :],
                                        start=(mt == 0), stop=(mt == NM - 1))
                    rcp = rcp_pool.tile([128, 4], f32, tag="rcp")
                    nc.vector.reciprocal(
                        rcp[:], pz[:].rearrange("p (k c) -> p k c", c=65)[:, :, 64:65])
                    for ts in range(2):
                        tt = tci * 2 + ts
                        for hl in range(2):
                            nc.vector.tensor_scalar_mul(
                                ZSB[tt][:, hp * 128 + hl * 64: hp * 128 + (hl + 1) * 64],
                                pz[:, (ts * 2 + hl) * 65:(ts * 2 + hl) * 65 + 64],
                                rcp[:, ts * 2 + hl: ts * 2 + hl + 1])

                pending = None
                for tci in range(NTC):
                    for hp in range(2):
                        egroups = emit_scores(hp, tci)
                        if pending is not None:
                            emit_av_norm(*pending)
                        pending = (hp, tci, egroups)
                emit_av_norm(*pending)

            # ---- Phase D: O-projection tail ----
            with (
                tc.tile_pool(name="pzt", bufs=3, space="PSUM") as pzt_pool,
                tc.tile_pool(name="po", bufs=3, space="PSUM") as po_pool,
                tc.tile_pool(name="zt", bufs=4) as zt_pool,
                tc.tile_pool(name="ob", bufs=4) as ob_pool,
            ):
                for tt in range(NTT):
                    pzt = pzt_pool.tile([128, HL * DH], f16, tag="pzt")
                    for di in range(NDO):
                        nc.tensor.transpose(pzt[:, di * 128:(di + 1) * 128],
                                            ZSB[tt][:, di * 128:(di + 1) * 128], identg[:])
                    zT = zt_pool.tile([128, HL * DH], f16, tag="zT")
                    nc.vector.tensor_copy(zT[:], pzt[:])
                    po = po_pool.tile([128, D], f32, tag="po")
                    for di in range(NDO):
                        nc.tensor.matmul(po[:], zT[:, di * 128:(di + 1) * 128],
                                         wo_sb[di][:],
                                         start=(di == 0), stop=(di == NDO - 1))
                    osb = ob_pool.tile([128, D], f32, tag="osb")
                    nc.scalar.copy(osb[:], po[:])
                    nc.sync.dma_start(out_d[tt * 128:(tt + 1) * 128, :], osb[:])

    nc.finalize()
    return nc


def _get_nc():
    if "nc" not in _CACHE:
        _CACHE["nc"] = _build()
    return _CACHE["nc"]


def build_in_maps(x, mem, Wq, bq, Wk, bk, Wv, bv, Wo, bo, **kw):
    x16T = np.ascontiguousarray(
        np.asarray(x, np.float32).astype(np.float16).transpose(0, 2, 1))
    mem16T = np.ascontiguousarray(
        np.asarray(mem, np.float32).astype(np.float16).transpose(0, 2, 1))
    Wo32 = np.asarray(Wo, np.float32)
    _CACHE["bo_eff"] = (np.asarray(bo, np.float32)
                        + np.asarray(bv, np.float32) @ Wo32).reshape(1, D)
    Wq16 = np.asarray(Wq, np.float32).astype(np.float16)
    Wk16 = (np.asarray(Wk, np.float32) * KSCALE).astype(np.float16)
    Wv16 = np.asarray(Wv, np.float32).astype(np.float16)
    Wo16 = Wo32.astype(np.float16)
    bq32 = np.asarray(bq, np.float32)
    identg = np.eye(128, dtype=np.float16)
    in_maps = []
    W = HL * DH
    for c in range(N_CORES):
        b, g = c // 2, c % 2
        sl = slice(g * W, (g + 1) * W)
        in_maps.append({
            "x16T": x16T[b],
            "mem16T": mem16T[b],
            "Wq": np.ascontiguousarray(Wq16[:, sl]),
            "Wk": np.ascontiguousarray(Wk16[:, sl]),
            "Wv": np.ascontiguousarray(Wv16[:, sl]),
            "Wo": np.ascontiguousarray(Wo16[sl, :]),
            "bqT": np.ascontiguousarray(bq32[sl].reshape(NDO, 128).T),
            "identg": identg,
        })
    return in_maps


def assemble(results):
    out = np.empty((B, T, D), np.float32)
    bo_eff = _CACHE["bo_eff"]
    for b in range(B):
        np.add(results[2 * b]["out"], results[2 * b + 1]["out"], out=out[b])
        out[b] += bo_eff
    return out


def kernel(**inputs):
    from concourse.bass_utils import run_bass_kernel_spmd

    nc = _get_nc()
    in_maps = build_in_maps(**inputs)
    res = run_bass_kernel_spmd(nc, in_maps, list(range(N_CORES)))
    _CACHE["last_res"] = res
    return assemble(res.results)
